# revision 1
# baseline (speedup 1.0000x reference)
"""Trainium2 Bass kernel for nn_DiGCNLayerAtt (directed GCN layer with
adjacency-masked attention), distributed batch-parallel over 8 NeuronCores.

Math notes (validated against the reference to ~5e-7 in f32):
  reference:  u = h h^T / sqrt(H); s = softmax(u); delta = s * A
              att = delta / (rowsum(delta)+eps); split att into strict-upper
              (left), diag (self), strict-lower (right) parts; ctx = sum of
              the three masked matmuls against h@W_d^T+b_d; out = relu(LN(ctx)).
  The softmax denominator, the renormalization, and the row-max subtraction
  are all uniform positive per-row scalings of att, and LayerNorm is invariant
  to a positive per-row scaling of ctx. So the kernel computes just
      attu[n,m] = exp(u_raw[n,m] / 16) * A[n,m]
  and LayerNorms the unnormalized context. exp fits comfortably in f32
  (max |u_offdiag|/16 ~ 6, diag |h_n|^2/16 ~ 22).

  u is symmetric, so the PE produces it directly in [m, n] orientation
  (contraction index m on partitions), which is what the ctx matmul needs as
  its stationary operand. Only A needs transposing — done host-side during
  sharding, cast to f16 (A is 0/1, exact).

  The f16 datapath: exp values on the true diagonal (e^~16) overflow f16, so
  the diagonal of u is extracted (for the self term) and zeroed in PSUM
  before the exp; the diagonal entries of attu are then finite junk that the
  strict-triangular masks of the diagonal blocks null out.

Per-core layout (batch b on core b):
  phase A: hdir_d = h @ W_d^T + b_d for d in {l,s,r}  (f16 in SBUF)
  phase B: for each CHUNK-wide column chunk of attu^T, for each 128-row
           m-block: u-matmul (f16, f32 acc) -> exp (ACT) -> *A^T (DVE)
           -> ctx matmuls accumulating ctx[n, h] in PSUM (double-buffered
           across chunks; GPSIMD elementwise measured too slow to offload).
  phase C: per chunk: fused evacuation + self-term add + row-sum (DVE).
  phase D: LayerNorm stats (E[(x-mu)x] = var*256), sqrt+reciprocal+Newton,
           (x-mu)*inv_std, relu, DMA out.
"""
import sys

sys.path.insert(0, "/opt/trn_rl_repo")

import numpy as np

import concourse.bass as bass
import concourse.tile as tile
from concourse import bacc, mybir
from concourse import bass_utils
from concourse.bass_interp import get_hw_module

F32 = mybir.dt.float32
F16 = mybir.dt.float16
ALU = mybir.AluOpType
ACTF = mybir.ActivationFunctionType
AX = mybir.AxisListType

B, N, H = 8, 2048, 256
NT = N // 128          # 16 n/m tiles of 128
CHUNK = 512            # attu^T column-chunk width
NCHUNK = N // CHUNK    # 2
NS = CHUNK // 128      # 8 n-subtiles per chunk
TEMPER_INV = 1.0 / float(np.sqrt(H))  # 1/16
EPS_LN = 1e-12


def build_program(apply_ln: bool, repeat: int = 1):
    nc = bacc.Bacc("TRN2", target_bir_lowering=False, debug=False, num_devices=B)

    # ---- DRAM I/O ----
    hT_d = nc.dram_tensor("hT", [H, N], F16, kind="ExternalInput")
    AT_d = nc.dram_tensor("AT", [N, N], F16, kind="ExternalInput")
    W_d = {d: nc.dram_tensor(f"W{d}T", [H, H], F16, kind="ExternalInput")
           for d in "lsr"}
    bias_d = {d: nc.dram_tensor(f"b{d}_bc", [128, H], F32, kind="ExternalInput")
              for d in "lsr"}
    masklo_d = nc.dram_tensor("masklo", [128, 128], F16, kind="ExternalInput")
    maskup_d = nc.dram_tensor("maskup", [128, 128], F16, kind="ExternalInput")
    eye_d = nc.dram_tensor("eye", [128, 128], F32, kind="ExternalInput")
    noeye_d = nc.dram_tensor("noeye", [128, 128], F32, kind="ExternalInput")
    adiag_d = nc.dram_tensor("adiag", [128, NT], F32, kind="ExternalInput")
    if apply_ln:
        lnw_d = nc.dram_tensor("lnw_bc", [128, H], F32, kind="ExternalInput")
        lnb_d = nc.dram_tensor("lnb_bc", [128, H], F32, kind="ExternalInput")
    out_d = nc.dram_tensor("out", [N, H], F32, kind="ExternalOutput")

    with tile.TileContext(nc) as tc:
        with (
            tc.tile_pool(name="consts", bufs=1) as cpool,
            tc.tile_pool(name="stream", bufs=6) as spool,
            tc.tile_pool(name="atp", bufs=NT) as atpool,
            tc.tile_pool(name="estream", bufs=4) as epool,
            tc.tile_pool(name="small", bufs=4) as smpool,
            tc.tile_pool(name="outp", bufs=3) as opool,
            tc.tile_pool(name="upsum", bufs=2, space=bass.MemorySpace.PSUM) as upool,
            tc.tile_pool(name="ctxpsum", bufs=2, space=bass.MemorySpace.PSUM) as xpool,
        ):
            v = nc.vector
            sc = nc.scalar

            # ---- constants / persistent SBUF ----
            hT0 = cpool.tile([128, N], F16, tag="hT0")
            hT1 = cpool.tile([128, N], F16, tag="hT1")
            nc.sync.dma_start(hT0[:], hT_d.ap()[0:128, :])
            nc.sync.dma_start(hT1[:], hT_d.ap()[128:256, :])
            wt = {}
            for d in "lsr":
                for k in (0, 1):
                    t = cpool.tile([128, H], F16, tag=f"W{d}T{k}")
                    nc.sync.dma_start(t[:], W_d[d].ap()[k * 128:(k + 1) * 128, :])
                    wt[d, k] = t
            bias_cat = cpool.tile([128, 3 * H], F32, tag="bias_cat")
            for i, d in enumerate("lsr"):
                nc.sync.dma_start(bias_cat[:, i * H:(i + 1) * H], bias_d[d].ap())
            masklo = cpool.tile([128, 128], F16, tag="masklo")
            maskup = cpool.tile([128, 128], F16, tag="maskup")
            eye = cpool.tile([128, 128], F32, tag="eye")
            noeye = cpool.tile([128, 128], F32, tag="noeye")
            adiag = cpool.tile([128, NT], F32, tag="adiag")
            nc.sync.dma_start(masklo[:], masklo_d.ap())
            nc.sync.dma_start(maskup[:], maskup_d.ap())
            nc.sync.dma_start(eye[:], eye_d.ap())
            nc.sync.dma_start(noeye[:], noeye_d.ap())
            nc.sync.dma_start(adiag[:], adiag_d.ap())
            if apply_ln:
                lnw = cpool.tile([128, H], F32, tag="lnw")
                lnb = cpool.tile([128, H], F32, tag="lnb")
                nc.sync.dma_start(lnw[:], lnw_d.ap())
                nc.sync.dma_start(lnb[:], lnb_d.ap())

            hlsr = [cpool.tile([128, 3 * H], F16, tag=f"hlsr{m}", name=f"hlsr{m}")
                    for m in range(NT)]
            hl = [t[:, 0:H] for t in hlsr]
            hs = [t[:, H:2 * H] for t in hlsr]
            hr = [t[:, 2 * H:3 * H] for t in hlsr]
            ctx_sb = cpool.tile([128, NT * H], F32, tag="ctx_sb")
            u_diag = cpool.tile([128, NT], F32, tag="u_diag")
            sum_b = cpool.tile([128, NT], F32, tag="sum_b")
            mu_b = cpool.tile([128, NT], F32, tag="mu_b")
            var_b = cpool.tile([128, NT], F32, tag="var_b")

            # `repeat` re-runs the whole compute chain for precise HW
            # timing: T(repeat=2) - T(repeat=1) cancels launch overhead.
            def emit_all():
                at_tiles = {}

                # phase A per-m emitter: hdir = h @ Wd^T + bd; interleaved
                # into chunk 0's m-loop so the exp/mask pipeline starts
                # filling immediately instead of after 96 serial matmuls.
                def emit_hdir(m):
                    ms = bass.ts(m, 128)
                    hp = xpool.tile([128, 768], F32, tag="ctx", name=f"hp{m}")
                    # bank0: l at [0:256], s at [256:512]; bank1: r at [512:768]
                    nc.tensor.matmul(hp[:, 0:256], hT0[:, ms], wt["l", 0][:],
                                     start=True, stop=False)
                    nc.tensor.matmul(hp[:, 256:512], hT0[:, ms], wt["s", 0][:],
                                     start=False, stop=False)
                    nc.tensor.matmul(hp[:, 512:768], hT0[:, ms], wt["r", 0][:],
                                     start=True, stop=False)
                    nc.tensor.matmul(hp[:, 0:256], hT1[:, ms], wt["l", 1][:],
                                     start=False, stop=False)
                    nc.tensor.matmul(hp[:, 256:512], hT1[:, ms], wt["s", 1][:],
                                     start=False, stop=True)
                    nc.tensor.matmul(hp[:, 512:768], hT1[:, ms], wt["r", 1][:],
                                     start=False, stop=True)
                    v.tensor_tensor(hlsr[m][:], hp[:, 0:768], bias_cat[:],
                                    op=ALU.add)

                # ---- phase B + C: main attention loop ----
                # AT row-panels loaded once per m-block (used by both chunks),
                # issue alternating between the two HWDGE rings (SP / ACT) to
                # avoid serializing on one sequencer's DMA FIFO.
                for chunk in range(NCHUNK):
                    c0 = chunk * CHUNK
                    ctx = xpool.tile([128, NS * H], F32, tag="ctx")  # 2 banks

                    def emit_ctx(m, attu, alo, aup, ctx=ctx, chunk=chunk):
                        for ns in range(NS):
                            nt = chunk * NS + ns
                            first = (m == 0) and (ns % 2 == 0)
                            last = (m == NT - 1) and (ns % 2 == 1)
                            o = ctx[:, ns * H:(ns + 1) * H]
                            if m > nt:      # m > n: left part
                                nc.tensor.matmul(o, attu[:, bass.ts(ns, 128)],
                                                 hl[m][:], start=first, stop=last)
                            elif m < nt:    # m < n: right part
                                nc.tensor.matmul(o, attu[:, bass.ts(ns, 128)],
                                                 hr[m][:], start=first, stop=last)
                            else:           # diagonal block: strict triangles
                                nc.tensor.matmul(o, alo[:], hl[m][:],
                                                 start=first, stop=False)
                                nc.tensor.matmul(o, aup[:], hr[m][:],
                                                 start=False, stop=last)

                    for m in range(NT):
                        ms = bass.ts(m, 128)
                        if chunk == 0:
                            at_full = atpool.tile([128, N], F16, tag="at",
                                                  name=f"at_{m}")
                            eng = nc.sync if m % 2 == 0 else nc.scalar
                            eng.dma_start(at_full[:], AT_d.ap()[ms, :])
                            at_tiles[m] = at_full
                        at = at_tiles[m][:, c0:c0 + CHUNK]
                        u = upool.tile([128, CHUNK], F32, tag="u", bufs=4)
                        for off in range(0, CHUNK, 512):
                            w = min(512, CHUNK - off)
                            nc.tensor.matmul(u[:, off:off + w], hT0[:, ms],
                                             hT0[:, c0 + off:c0 + off + w],
                                             start=True, stop=False)
                            nc.tensor.matmul(u[:, off:off + w], hT1[:, ms],
                                             hT1[:, c0 + off:c0 + off + w],
                                             start=False, stop=True)
                        if chunk == 0:
                            emit_hdir(m)
                        is_diag = (m // NS) == chunk
                        alo = aup = None
                        if is_diag:
                            dc = (m % NS) * 128
                            dtmp = smpool.tile([128, 128], F32, tag="dtmp")
                            v.tensor_tensor(dtmp[:], u[:, dc:dc + 128], eye[:],
                                            op=ALU.mult)
                            v.reduce_sum(u_diag[:, m:m + 1], dtmp[:], axis=AX.X)
                            # zero the diagonal so exp stays in f16 range
                            v.tensor_tensor(u[:, dc:dc + 128], u[:, dc:dc + 128],
                                            noeye[:], op=ALU.mult)
                        e = epool.tile([128, CHUNK], F16, tag="e")
                        sc.activation(e[:], u[:], ACTF.Exp, scale=TEMPER_INV)
                        attu = spool.tile([128, CHUNK], F16, tag="attu")
                        v.tensor_tensor(attu[:], e[:], at[:], op=ALU.mult)
                        if is_diag:
                            alo = smpool.tile([128, 128], F16, tag="alo")
                            aup = smpool.tile([128, 128], F16, tag="aup")
                            v.tensor_tensor(alo[:], attu[:, dc:dc + 128],
                                            masklo[:], op=ALU.mult)
                            v.tensor_tensor(aup[:], attu[:, dc:dc + 128],
                                            maskup[:], op=ALU.mult)
                        emit_ctx(m, attu, alo, aup)

                    # phase C: evacuate ctx + self term + row sums
                    de = smpool.tile([128, NS], F32, tag="de")
                    sc.activation(de[:], u_diag[:, chunk * NS:(chunk + 1) * NS],
                                  ACTF.Exp, scale=TEMPER_INV)
                    dv = smpool.tile([128, NS], F32, tag="dv")
                    v.tensor_tensor(dv[:], de[:],
                                    adiag[:, chunk * NS:(chunk + 1) * NS],
                                    op=ALU.mult)
                    for ns in range(NS):
                        nt = chunk * NS + ns
                        tmpd = smpool.tile([128, H], F32, tag="tmpd")
                        v.tensor_scalar(tmpd[:], hs[nt][:], dv[:, ns:ns + 1],
                                        None, op0=ALU.mult)
                        cs = ctx_sb[:, nt * H:(nt + 1) * H]
                        v.scalar_tensor_tensor(cs, ctx[:, ns * H:(ns + 1) * H], 1.0,
                                               tmpd[:], op0=ALU.mult, op1=ALU.add,
                                               accum_out=sum_b[:, nt:nt + 1])
                        v.tensor_scalar(mu_b[:, nt:nt + 1], sum_b[:, nt:nt + 1],
                                        1.0 / H, None, op0=ALU.mult)
                        sq = smpool.tile([128, H], F32, tag="sq")
                        v.scalar_tensor_tensor(sq[:], cs, mu_b[:, nt:nt + 1], cs,
                                               op0=ALU.subtract, op1=ALU.mult,
                                               accum_out=var_b[:, nt:nt + 1])

                # ---- phase D: LayerNorm + relu + store ----
                veps = smpool.tile([128, NT], F32, tag="veps")
                v.tensor_scalar(veps[:], var_b[:], 1.0 / H, EPS_LN,
                                op0=ALU.mult, op1=ALU.add)
                sig = smpool.tile([128, NT], F32, tag="sig")
                sc.activation(sig[:], veps[:], ACTF.Sqrt)
                inv0 = smpool.tile([128, NT], F32, tag="inv0")
                v.reciprocal(inv0[:], sig[:])
                # one Newton step: istd = inv0 * (1.5 - 0.5 * veps * inv0^2)
                nw1 = smpool.tile([128, NT], F32, tag="nw1")
                v.tensor_tensor(nw1[:], inv0[:], inv0[:], op=ALU.mult)
                v.tensor_tensor(nw1[:], veps[:], nw1[:], op=ALU.mult)
                v.tensor_scalar(nw1[:], nw1[:], -0.5, 1.5, op0=ALU.mult, op1=ALU.add)
                istd = smpool.tile([128, NT], F32, tag="istd")
                v.tensor_tensor(istd[:], inv0[:], nw1[:], op=ALU.mult)
                out_sb = cpool.tile([128, NT * H], F32, tag="out_sb")
                # 4-way store: DRAM [2048, 256] viewed as [p, t, h]; each
                # group of 4 n-tiles leaves as soon as its relu is done
                out_v = out_d.ap().rearrange("(t p) h -> p t h", p=128)
                out_sv = out_sb[:].rearrange("p (t h) -> p t h", h=H)
                for nt in range(NT):
                    pre = opool.tile([128, H], F32, tag="pre")
                    v.tensor_scalar(pre[:], ctx_sb[:, nt * H:(nt + 1) * H],
                                    mu_b[:, nt:nt + 1], istd[:, nt:nt + 1],
                                    op0=ALU.subtract, op1=ALU.mult)
                    if apply_ln:
                        v.tensor_tensor(pre[:], pre[:], lnw[:], op=ALU.mult)
                        v.tensor_tensor(pre[:], pre[:], lnb[:], op=ALU.add)
                    sc.activation(out_sb[:, nt * H:(nt + 1) * H], pre[:], ACTF.Relu)
                    if nt % 4 == 3:
                        g = nt // 4
                        eng = nc.sync if g % 2 == 0 else nc.scalar
                        eng.dma_start(out_v[:, g * 4:(g + 1) * 4, :],
                                      out_sv[:, g * 4:(g + 1) * 4, :])

            for _rep in range(repeat):
                emit_all()

    nc.compile()
    nc.m = get_hw_module(nc.m)
    return nc


_cache = {}


def _get_program(apply_ln: bool):
    if apply_ln not in _cache:
        _cache[apply_ln] = build_program(apply_ln)
    return _cache[apply_ln]


def _prep_in_maps(hidden_state, adjacency_matrix, Wl, bl, Ws, bs, Wr, br,
                  ln_w, ln_b, apply_ln):
    f16 = np.float16
    tri = np.tri(128, 128, -1)
    shared = {
        "WlT": np.ascontiguousarray(Wl.T).astype(f16),
        "WsT": np.ascontiguousarray(Ws.T).astype(f16),
        "WrT": np.ascontiguousarray(Wr.T).astype(f16),
        "bl_bc": np.broadcast_to(bl.astype(np.float32), (128, H)).copy(),
        "bs_bc": np.broadcast_to(bs.astype(np.float32), (128, H)).copy(),
        "br_bc": np.broadcast_to(br.astype(np.float32), (128, H)).copy(),
        "masklo": tri.astype(f16),                 # [p,q]: 1 if p>q (m>n: left)
        "maskup": tri.T.astype(f16),               # 1 if p<q (m<n: right)
        "eye": np.eye(128, dtype=np.float32),
        "noeye": (1.0 - np.eye(128)).astype(np.float32),
    }
    if apply_ln:
        shared["lnw_bc"] = np.broadcast_to(ln_w.astype(np.float32), (128, H)).copy()
        shared["lnb_bc"] = np.broadcast_to(ln_b.astype(np.float32), (128, H)).copy()
    A16 = adjacency_matrix.astype(f16)
    h16 = hidden_state.astype(f16)
    in_maps = []
    for b in range(B):
        diag = np.diagonal(adjacency_matrix[b]).astype(np.float32)
        m = dict(shared)
        m["hT"] = np.ascontiguousarray(h16[b].T)
        m["AT"] = np.ascontiguousarray(A16[b].T)
        m["adiag"] = np.ascontiguousarray(diag.reshape(NT, 128).T)
        in_maps.append(m)
    return in_maps


def kernel(hidden_state, adjacency_matrix, Wl, bl, Ws, bs, Wr, br, ln_w, ln_b):
    apply_ln = not (np.all(ln_w == 1.0) and np.all(ln_b == 0.0))
    nc = _get_program(apply_ln)
    in_maps = _prep_in_maps(hidden_state, adjacency_matrix, Wl, bl, Ws, bs,
                            Wr, br, ln_w, ln_b, apply_ln)
    res = bass_utils.run_bass_kernel_spmd(nc, in_maps, core_ids=list(range(B)))
    return np.stack([res.results[b]["out"] for b in range(B)]).astype(np.float32)

